# revision 17
# baseline (speedup 1.0000x reference)
import sys
sys.path.insert(0, "/opt/trn_rl_repo")

import numpy as np
import ml_dtypes
from contextlib import ExitStack

import concourse.bass as bass
import concourse.bacc as bacc_mod
import concourse.tile as tile
import concourse.mybir as mybir
from concourse.alu_op_type import AluOpType
from concourse.bass_utils import run_bass_kernel_spmd

BF16 = mybir.dt.bfloat16
F32 = mybir.dt.float32
AF = mybir.ActivationFunctionType
AX = mybir.AxisListType

B, CIN, H, W = 4, 16, 256, 256
SHIFTS = (1, 2, 4, 8)
NS = 4          # shift heads
NH = 4          # attention heads
HID = 16
USE_COLLECTIVE = True
ROWS = 128 if USE_COLLECTIVE else 256   # rows per core
A = ROWS * W
CH = 2048       # free-dim chunk for DMA staging
NCH = A // CH
NT1 = A // 128  # pass-1 subtiles
EPS_IN = 1e-5

_OFFS = [(-1, -1), (-1, 0), (-1, 1), (0, -1), (0, 1), (1, -1), (1, 0), (1, 1)]


def _build_program():
    nc = bacc_mod.Bacc("TRN2", target_bir_lowering=False, debug=False, num_devices=8)
    sur = nc.dram_tensor("sur", [NS, 128, ROWS, W], BF16, kind="ExternalInput")
    cen = nc.dram_tensor("cen", [CIN, ROWS, W], BF16, kind="ExternalInput")
    wk = nc.dram_tensor("wk", [NS, 128, 128], BF16, kind="ExternalInput")
    wv = nc.dram_tensor("wv", [NS, 128, 128], BF16, kind="ExternalInput")
    wq = nc.dram_tensor("wq", [CIN, 64], BF16, kind="ExternalInput")
    wo = nc.dram_tensor("wo", [64, 16], BF16, kind="ExternalInput")
    bnb = nc.dram_tensor("bnb", [16, 1], F32, kind="ExternalInput")
    onesblk = nc.dram_tensor("onesblk", [64, 4], BF16, kind="ExternalInput")
    ident = nc.dram_tensor("ident", [64, 64], F32, kind="ExternalInput")
    oblkt = nc.dram_tensor("oblkt", [4, 64], F32, kind="ExternalInput")
    out = nc.dram_tensor("out", [16, ROWS, W], F32, kind="ExternalOutput")

    if USE_COLLECTIVE:
        pmask = nc.dram_tensor("pmask", [65, 8], F32, kind="ExternalInput")

    sur_f = sur.rearrange("s g r w -> s g (r w)")
    cen_f = cen.rearrange("c r w -> c (r w)")
    out_f = out.rearrange("c r w -> c (r w)")

    with tile.TileContext(nc) as tc, ExitStack() as ctx:
        singles = ctx.enter_context(tc.tile_pool(name="singles", bufs=1))
        xg_p = ctx.enter_context(tc.tile_pool(name="xg", bufs=8))
        cen_p = ctx.enter_context(tc.tile_pool(name="cen", bufs=2))
        kq_p = ctx.enter_context(tc.tile_pool(name="kq", bufs=3))
        sq_p = ctx.enter_context(tc.tile_pool(name="sq", bufs=3))
        stp = ctx.enter_context(tc.tile_pool(name="stats", bufs=1))
        vsb_p = ctx.enter_context(tc.tile_pool(name="vsb", bufs=6))
        osb_p = ctx.enter_context(tc.tile_pool(name="osb", bufs=2))
        fout_p = ctx.enter_context(tc.tile_pool(name="fout", bufs=3))
        ps1 = ctx.enter_context(ExitStack())
        ps_work = ps1.enter_context(tc.tile_pool(name="psw", bufs=2, space="PSUM"))
        ps_acc = ps1.enter_context(tc.tile_pool(name="psa", bufs=1, space="PSUM"))

        # weights to SBUF
        wk_sb = [singles.tile([128, 128], BF16, tag=f"wk{s}", name=f"wk_sb{s}") for s in range(NS)]
        wv_sb = [singles.tile([128, 128], BF16, tag=f"wv{s}", name=f"wv_sb{s}") for s in range(NS)]
        for s in range(NS):
            nc.gpsimd.dma_start(out=wk_sb[s], in_=wk[s])
            nc.gpsimd.dma_start(out=wv_sb[s], in_=wv[s])
        wq_sb = singles.tile([CIN, 64], BF16)
        nc.gpsimd.dma_start(out=wq_sb, in_=wq[:])
        wo_sb = singles.tile([64, 16], BF16)
        nc.gpsimd.dma_start(out=wo_sb, in_=wo[:])
        bnb_sb = singles.tile([16, 1], F32)
        nc.gpsimd.dma_start(out=bnb_sb, in_=bnb[:])
        oblk_sb = singles.tile([64, 4], BF16)
        nc.gpsimd.dma_start(out=oblk_sb, in_=onesblk[:])
        id_sb = singles.tile([64, 64], F32)
        nc.gpsimd.dma_start(out=id_sb, in_=ident[:])
        oblkt_sb = singles.tile([4, 64], F32)
        nc.gpsimd.dma_start(out=oblkt_sb, in_=oblkt[:])
        ones128 = singles.tile([128, 1], BF16)
        nc.vector.memset(ones128, 1.0)

        # persistent accumulators
        sc_acc = ps_acc.tile([64, 512], F32)    # scores: [64 qcols, 4s*128 kcols]
        kn_acc = ps_acc.tile([1, 512], F32)
        qn_acc = ps_acc.tile([1, 64], F32)

        # ---------------- pass 1: K,Q conv + scores + norms ----------------
        for ch in range(NCH):
            xg = []
            for s in range(NS):
                t = xg_p.tile([128, CH], BF16, tag=f"xg{s}", name=f"xgt{s}")
                nc.sync.dma_start(out=t, in_=sur_f[s, :, ch * CH:(ch + 1) * CH])
                xg.append(t)
            cen_t = cen_p.tile([CIN, CH], BF16)
            nc.sync.dma_start(out=cen_t, in_=cen_f[:, ch * CH:(ch + 1) * CH])
            for u in range(CH // 128):
                t = ch * (CH // 128) + u
                first = t == 0
                last = t == NT1 - 1
                kp = ps_work.tile([128, 512], F32, tag="kp")
                for s in range(NS):
                    nc.tensor.matmul(kp[:, s * 128:(s + 1) * 128],
                                     lhsT=xg[s][:, u * 128:(u + 1) * 128],
                                     rhs=wk_sb[s], start=True, stop=True)
                qp = ps_work.tile([128, 64], F32, tag="qp")
                nc.tensor.matmul(qp, lhsT=cen_t[:, u * 128:(u + 1) * 128],
                                 rhs=wq_sb, start=True, stop=True)
                kq = kq_p.tile([128, 576], BF16)
                nc.scalar.copy(kq[:, 0:512], kp)
                nc.scalar.copy(kq[:, 512:576], qp)
                sq = sq_p.tile([128, 576], BF16)
                nc.vector.tensor_mul(sq, kq, kq)
                for s in range(NS):
                    nc.tensor.matmul(sc_acc[:, s * 128:(s + 1) * 128],
                                     lhsT=kq[:, 512:576],
                                     rhs=kq[:, s * 128:(s + 1) * 128],
                                     start=(first and s == 0), stop=last,
                                     skip_group_check=True)
                nc.tensor.matmul(kn_acc, lhsT=ones128, rhs=sq[:, 0:512],
                                 start=first, stop=last, skip_group_check=True)
                nc.tensor.matmul(qn_acc, lhsT=ones128, rhs=sq[:, 512:576],
                                 start=first, stop=last, skip_group_check=True)

        # ---------------- stats: allreduce + attn weights ----------------
        sc_sb = stp.tile([65, 576], F32)
        nc.vector.memset(sc_sb, 0.0)
        nc.scalar.copy(sc_sb[0:64, 0:512], sc_acc)
        nc.scalar.copy(sc_sb[64:65, 0:512], kn_acc)
        nc.scalar.copy(sc_sb[64:65, 512:576], qn_acc)

        if USE_COLLECTIVE:
            pm_sb = stp.tile([65, 8], F32)
            nc.gpsimd.dma_start(out=pm_sb, in_=pmask[:])
            sti_sb = stp.tile([65, 8, 576], F32)
            for c in range(8):
                nc.vector.tensor_scalar_mul(sti_sb[:, c, :], sc_sb, pm_sb[:, c:c + 1])
            stats_full = stp.tile([65, 576], F32)
            dramp = ctx.enter_context(tc.tile_pool(name="dramp", bufs=1, space="DRAM"))
            st_in = dramp.tile([8, 65, 576], F32)
            st_out = dramp.tile([65, 576], F32)
            nc.gpsimd.dma_start(out=st_in.rearrange("s p f -> p s f"), in_=sti_sb)
            nc.gpsimd.collective_compute(
                "ReduceScatter", AluOpType.add,
                replica_groups=[[0, 1, 2, 3, 4, 5, 6, 7]],
                ins=[st_in.opt()], outs=[st_out.opt()])
            nc.gpsimd.dma_start(out=stats_full, in_=st_out[:])
        else:
            stats_full = sc_sb

        sc_raw = stats_full[0:64, 0:512]
        kn_v = stats_full[64:65, 0:512]
        qn_v = stats_full[64:65, 512:576]

        rsq = stp.tile([1, 576], F32)
        sqt = stp.tile([1, 576], F32)
        nc.scalar.activation(sqt[:, 0:512], kn_v, AF.Sqrt)
        nc.scalar.activation(sqt[:, 512:576], qn_v, AF.Sqrt, scale=float(H * W))
        nc.vector.reciprocal(rsq, sqt)
        outer_ps = ps_work.tile([64, 512], F32, tag="stx", bufs=1)
        nc.tensor.matmul(outer_ps, lhsT=rsq[:, 512:576], rhs=rsq[:, 0:512],
                         start=True, stop=True)
        outer_sb = stp.tile([64, 512], F32)
        nc.scalar.copy(outer_sb, outer_ps)
        scn = stp.tile([64, 512], F32)
        nc.vector.tensor_mul(scn, sc_raw, outer_sb)

        # gather per-head blocks: sc_g[16h2+c, s*32+j] = scn[16h2+c, s*128+32*h2+j]
        sc_g = stp.tile([64, 128], F32)
        for h2 in range(NH):
            for s in range(NS):
                nc.sync.dma_start(
                    out=sc_g[16 * h2:16 * (h2 + 1), 32 * s:32 * (s + 1)],
                    in_=scn[16 * h2:16 * (h2 + 1),
                            128 * s + 32 * h2:128 * s + 32 * h2 + 32])

        # instance-norm stats per head over [16,128] block
        sc_gb = stp.tile([64, 128], BF16)
        nc.vector.tensor_copy(sc_gb, sc_g)
        sq_gb = stp.tile([64, 128], BF16)
        nc.vector.tensor_mul(sq_gb, sc_gb, sc_gb)
        mps = ps_work.tile([4, 256], F32, tag="stx", bufs=1, name="mps")
        nc.tensor.matmul(mps[:, 0:128], lhsT=oblk_sb, rhs=sc_gb, start=True, stop=True)
        nc.tensor.matmul(mps[:, 128:256], lhsT=oblk_sb, rhs=sq_gb, start=True, stop=True)
        msums = stp.tile([4, 256], F32)
        nc.scalar.copy(msums, mps)
        sums = stp.tile([4, 2], F32)
        nc.vector.reduce_sum(sums[:, 0:1], msums[:, 0:128], axis=AX.X)
        nc.vector.reduce_sum(sums[:, 1:2], msums[:, 128:256], axis=AX.X)
        mv2 = stp.tile([4, 2], F32)
        nc.scalar.mul(mv2[:, 0:1], sums[:, 0:1], 1.0 / 2048.0)
        nc.scalar.mul(mv2[:, 1:2], sums[:, 1:2], 1.0 / 2048.0)
        m2 = stp.tile([4, 1], F32)
        nc.vector.tensor_mul(m2, mv2[:, 0:1], mv2[:, 0:1])
        var = stp.tile([4, 1], F32)
        nc.vector.tensor_sub(var, mv2[:, 1:2], m2)
        sdt = stp.tile([4, 1], F32)
        epst = stp.tile([4, 1], F32)
        nc.vector.memset(epst, EPS_IN)
        nc.scalar.activation(sdt, var, AF.Sqrt, bias=epst)
        nc.vector.reciprocal(mv2[:, 1:2], sdt)
        bc_ps = ps_work.tile([64, 2], F32, tag="stx", bufs=1, name="bc_ps")
        nc.tensor.matmul(bc_ps, lhsT=oblkt_sb, rhs=mv2, start=True, stop=True)
        bc_sb = stp.tile([64, 2], F32)
        nc.scalar.copy(bc_sb, bc_ps)
        mean_bc = bc_sb[:, 0:1]
        rstd_bc = bc_sb[:, 1:2]

        t0 = stp.tile([64, 128], F32)
        nc.vector.tensor_scalar_sub(t0, sc_g, mean_bc)
        ex = stp.tile([64, 128], F32)
        nc.scalar.activation(ex, t0, AF.Exp, scale=rstd_bc)
        rs_ = stp.tile([64, 1], F32)
        nc.vector.reduce_sum(rs_, ex, axis=AX.X)
        rr = stp.tile([64, 1], F32)
        nc.vector.reciprocal(rr, rs_)
        attn = stp.tile([64, 128], F32)
        nc.vector.tensor_scalar_mul(attn, ex, rr)

        atp = ps_work.tile([128, 64], F32, tag="stx", bufs=1, name="atp")
        nc.tensor.transpose(atp, attn, id_sb)
        attnT = stp.tile([128, 64], F32)
        nc.scalar.copy(attnT, atp)
        aw = []
        for s in range(NS):
            w = stp.tile([128, 64], BF16, tag=f"aw{s}", name=f"awt{s}")
            nc.vector.memset(w, 0.0)
            for h2 in range(NH):
                nc.vector.tensor_copy(
                    w[32 * h2:32 * h2 + 32, 16 * h2:16 * h2 + 16],
                    attnT[32 * s:32 * s + 32, 16 * h2:16 * h2 + 16])
            aw.append(w)

        # ---------------- pass 2: V conv + attn@V + outconv + BN/ReLU ----------------
        ps1.close()
        ps2 = ctx.enter_context(tc.tile_pool(name="ps2", bufs=2, space="PSUM"))
        for ch in range(NCH):
            xg = []
            for s in range(NS):
                t = xg_p.tile([128, CH], BF16, tag=f"xg{s}", name=f"xgt{s}")
                nc.sync.dma_start(out=t, in_=sur_f[s, :, ch * CH:(ch + 1) * CH])
                xg.append(t)
            for q in range(CH // 512):
                fs = 512 * q
                op = ps2.tile([64, 512], F32, tag="op")
                for s in range(NS):
                    vp = ps2.tile([128, 512], F32, tag="vp")
                    nc.tensor.matmul(vp, lhsT=wv_sb[s], rhs=xg[s][:, fs:fs + 512],
                                     start=True, stop=True)
                    vsb = vsb_p.tile([128, 512], BF16)
                    nc.vector.tensor_copy(vsb, vp)
                    nc.tensor.matmul(op, lhsT=aw[s], rhs=vsb,
                                     start=(s == 0), stop=(s == 3))
                osb = osb_p.tile([64, 512], BF16)
                nc.scalar.copy(osb, op)
                fp = ps2.tile([16, 512], F32, tag="fp")
                nc.tensor.matmul(fp, lhsT=wo_sb, rhs=osb, start=True, stop=True)
                fout = fout_p.tile([16, 512], F32)
                nc.scalar.activation(fout, fp, AF.Relu, bias=bnb_sb)
                nc.sync.dma_start(out=out_f[:, ch * CH + fs:ch * CH + fs + 512],
                                  in_=fout)
    return nc


_NC = None


def _get_nc():
    global _NC
    if _NC is None:
        _NC = _build_program()
        if not _NC.is_finalized():
            _NC.finalize()
    return _NC


def kernel(cen, q_w, k_w, v_w, out_w, bn_gamma, bn_beta, bn_mean, bn_var):
    bf = ml_dtypes.bfloat16
    pad = np.pad(cen, ((0, 0), (0, 0), (8, 8), (8, 8)), mode="reflect")  # [B,16,272,272]

    scale = bn_gamma / np.sqrt(bn_var + 1e-5)
    wo_np = (out_w * scale[:, None]).T.astype(bf)          # [64,16]
    bnb_np = (bn_beta - bn_mean * scale)[:, None].astype(np.float32)
    wq_np = np.zeros((CIN, 64), np.float32)
    for h2 in range(NH):
        for o in range(4):
            for s in range(NS):
                wq_np[:, 16 * h2 + o * 4 + s] = q_w[s, 4 * h2 + o, :]
    wq_np = wq_np.astype(bf)
    wk_np = np.ascontiguousarray(np.transpose(k_w, (0, 2, 1))).astype(bf)  # [s,128in,128out]
    wv_np = np.ascontiguousarray(np.transpose(v_w, (0, 2, 1))).astype(bf)
    oblk = np.zeros((64, 4), np.float32)
    for h2 in range(NH):
        oblk[16 * h2:16 * (h2 + 1), h2] = 1.0
    oblk = oblk.astype(bf)
    ident = np.eye(64, dtype=np.float32)

    n_cores = 8 if USE_COLLECTIVE else 8
    in_maps = []
    for core in range(n_cores):
        if USE_COLLECTIVE:
            b, half = core // 2, core % 2
            base = half * 128
        else:
            b, base = core % B, 0
        p = pad[b]  # [16, 272, 272]
        cen_loc = p[:, 8 + base:8 + base + ROWS, 8:8 + W]
        sur = np.empty((NS, 128, ROWS, W), bf)
        for s, d in enumerate(SHIFTS):
            for j, (dy, dx) in enumerate(_OFFS):
                sh = p[:, 8 + base + dy * d:8 + base + dy * d + ROWS,
                       8 + dx * d:8 + dx * d + W]
                sur[s, 16 * j:16 * (j + 1)] = (sh - cen_loc).astype(bf)
        pm = np.zeros((65, 8), np.float32)
        pm[:, 2 * (core // 2):2 * (core // 2) + 2] = 1.0
        in_maps.append(dict(
            sur=sur, cen=cen_loc.astype(bf), wk=wk_np, wv=wv_np, wq=wq_np,
            wo=wo_np, bnb=bnb_np, onesblk=oblk, ident=ident, pmask=pm,
            oblkt=np.ascontiguousarray(oblk.astype(np.float32).T)))

    res = run_bass_kernel_spmd(_get_nc(), in_maps, list(range(n_cores))).results

    out = np.empty((B, 16, H, W), np.float32)
    if USE_COLLECTIVE:
        for core in range(8):
            b, half = core // 2, core % 2
            out[b, :, half * 128:half * 128 + 128, :] = (
                res[core]["out"].reshape(16, ROWS, W))
    else:
        for b in range(B):
            out[b] = res[b]["out"].reshape(16, ROWS, W)
    return out
